# revision 3
# baseline (speedup 1.0000x reference)
"""Trainium kernel for nn_CombinedModel_Av2 (audio -> mel -> CNN -> enc/dec transformer -> greedy decode).

Strategy: SPMD data-parallel over batch (B=4) across 4 NeuronCores via pmap.
Algorithmic improvements vs reference: encoder runs once (reference re-runs it
every decode step), decoder cross-attention K/V cached, rfft replaced by a
windowed DFT matmul (PE-friendly).
Self-contained: hardcodes all shapes; no sibling imports.
"""
import numpy as np

N_FFT, HOP, N_MELS, SR = 2048, 160, 229, 16000
D, H, FFN, L, NC, CH = 256, 8, 1024, 2, 39, 48
EPS = 1e-10
B, SAMPLES, TT = 4, 64000, 8
NFRAMES = SAMPLES // HOP + 1  # 401
F_OUT = 14
DH = D // H

_jax = None
_fwd = None


def _dft_consts():
    n = np.arange(N_FFT, dtype=np.float64)
    k = np.arange(N_FFT // 2 + 1, dtype=np.float64)
    ang = -2.0 * np.pi * np.outer(n, k) / N_FFT
    w = 0.5 - 0.5 * np.cos(2.0 * np.pi * n / N_FFT)
    cos = (np.cos(ang) * w[:, None]).astype(np.float32)  # (2048, 1025)
    sin = (np.sin(ang) * w[:, None]).astype(np.float32)
    return cos, sin


def _sinpos(T, Dm):
    pos = np.arange(T, dtype=np.float32)[:, None]
    div = np.exp(np.arange(0, Dm, 2, dtype=np.float32) * (-np.log(10000.0) / Dm))
    pe = np.zeros((T, Dm), np.float32)
    pe[:, 0::2] = np.sin(pos * div)
    pe[:, 1::2] = np.cos(pos * div)
    return pe


def _build():
    global _jax, _fwd
    if _fwd is not None:
        return
    import jax
    import jax.numpy as jnp
    from jax import lax
    _jax = jax

    COS, SIN = _dft_consts()
    PE = _sinpos(NFRAMES, D)
    FRAME_IDX = (np.arange(NFRAMES, dtype=np.int32)[:, None] * HOP
                 + np.arange(N_FFT, dtype=np.int32)[None, :])

    def layer_norm(x, gb):
        m = x.mean(-1, keepdims=True)
        v = ((x - m) ** 2).mean(-1, keepdims=True)
        return (x - m) / jnp.sqrt(v + 1e-5) * gb[0] + gb[1]

    def mha(xq, xkv, w, b, mask=None):
        Bq, Sq, Dm = xq.shape
        q = (xq @ w[0] + b[0]).reshape(Bq, Sq, H, DH)
        k = (xkv @ w[1] + b[1]).reshape(Bq, -1, H, DH)
        v = (xkv @ w[2] + b[2]).reshape(Bq, -1, H, DH)
        s = jnp.einsum('bqhd,bkhd->bhqk', q, k) / jnp.sqrt(jnp.float32(DH))
        if mask is not None:
            s = jnp.where(mask, s, -1e9)
        a = jax.nn.softmax(s, axis=-1)
        o = jnp.einsum('bhqk,bkhd->bqhd', a, v).reshape(Bq, Sq, Dm)
        return o @ w[3] + b[3]

    def mha_cached(xq, kc, vc, wq, bq, wo, bo):
        # cross-attention with precomputed K/V caches (Bq, S, H, DH)
        Bq, Sq, Dm = xq.shape
        q = (xq @ wq + bq).reshape(Bq, Sq, H, DH)
        s = jnp.einsum('bqhd,bkhd->bhqk', q, kc) / jnp.sqrt(jnp.float32(DH))
        a = jax.nn.softmax(s, axis=-1)
        o = jnp.einsum('bhqk,bkhd->bqhd', a, vc).reshape(Bq, Sq, Dm)
        return o @ wo + bo

    def forward(waveform, mel_fb, window, conv1_w, conv1_b, conv2_w, conv2_b,
                proj_w, proj_b, query_embed, cls_w, cls_b,
                enc_attn_w, enc_attn_b, enc_ln, enc_ff_w1, enc_ff_b1,
                enc_ff_w2, enc_ff_b2,
                dec_sa_w, dec_sa_b, dec_ca_w, dec_ca_b, dec_ln,
                dec_ff_w1, dec_ff_b1, dec_ff_w2, dec_ff_b2):
        # waveform: (1, 64000) one batch element per device
        pad = N_FFT // 2
        wp = jnp.pad(waveform, ((0, 0), (pad, pad)), mode='reflect')
        frames = wp[:, FRAME_IDX]                      # (1, 401, 2048) window folded into DFT
        re = frames @ COS
        im = frames @ SIN
        power = re * re + im * im                      # (1, 401, 1025)
        x = jnp.log(power @ mel_fb + EPS)              # (1, 401, 229)
        spec = x[:, None]
        dn = ('NCHW', 'OIHW', 'NCHW')
        h = jax.nn.relu(lax.conv_general_dilated(spec, conv1_w, (1, 1), 'SAME',
                                                 dimension_numbers=dn)
                        + conv1_b[None, :, None, None])
        h = lax.reduce_window(h, -jnp.inf, lax.max, (1, 1, 1, 4), (1, 1, 1, 4), 'VALID')
        h = jax.nn.relu(lax.conv_general_dilated(h, conv2_w, (1, 1), 'SAME',
                                                 dimension_numbers=dn)
                        + conv2_b[None, :, None, None])
        h = lax.reduce_window(h, -jnp.inf, lax.max, (1, 1, 1, 4), (1, 1, 1, 4), 'VALID')
        backbone_feat = h                              # (1, 48, 401, 14)
        B_, C_, T_, F_ = h.shape
        xf = jnp.swapaxes(h, -1, -2).reshape(B_, C_ * F_, T_)
        hseq = jnp.einsum('bct,cd->btd', xf, proj_w) + proj_b + PE[None]

        # ---- encoder, computed ONCE ----
        x_enc = hseq
        for l in range(L):
            x_enc = layer_norm(x_enc + mha(x_enc, x_enc, enc_attn_w[l], enc_attn_b[l]),
                               enc_ln[l, 0])
            f = jax.nn.relu(x_enc @ enc_ff_w1[l] + enc_ff_b1[l]) @ enc_ff_w2[l] + enc_ff_b2[l]
            x_enc = layer_norm(x_enc + f, enc_ln[l, 1])
        mem = x_enc

        # ---- cross-attention K/V caches, computed ONCE ----
        kc = [(mem @ dec_ca_w[l, 1] + dec_ca_b[l, 1]).reshape(1, -1, H, DH) for l in range(L)]
        vc = [(mem @ dec_ca_w[l, 2] + dec_ca_b[l, 2]).reshape(1, -1, H, DH) for l in range(L)]

        def decoder(tgt):
            t = tgt.shape[1]
            causal = jnp.tril(jnp.ones((t, t), bool))
            y = tgt
            for l in range(L):
                y = layer_norm(y + mha(y, y, dec_sa_w[l], dec_sa_b[l], causal),
                               dec_ln[l, 0])
                y = layer_norm(y + mha_cached(y, kc[l], vc[l],
                                              dec_ca_w[l, 0], dec_ca_b[l, 0],
                                              dec_ca_w[l, 3], dec_ca_b[l, 3]),
                               dec_ln[l, 1])
                f = jax.nn.relu(y @ dec_ff_w1[l] + dec_ff_b1[l]) @ dec_ff_w2[l] + dec_ff_b2[l]
                y = layer_norm(y + f, dec_ln[l, 2])
            return y @ cls_w + cls_b

        # ---- greedy decode, exact reference semantics ----
        seq = jnp.full((1, 1), NC, jnp.int32)
        so = decoder(query_embed[seq])
        out = so
        seq = jnp.concatenate([seq, jnp.argmax(so, -1).astype(jnp.int32)], axis=1)
        for _ in range(TT - 1):
            so = decoder(query_embed[seq])
            out = jnp.concatenate([out, so[:, -1:]], axis=1)
            seq = jnp.concatenate([seq, jnp.argmax(so[:, -1:], -1).astype(jnp.int32)], axis=1)
        return out, spec, backbone_feat

    global _raw_forward
    _raw_forward = forward
    devs = jax.devices()[:B]
    if len(devs) >= B:
        in_axes = (0,) + (None,) * 27
        _fwd = ('pmap', jax.pmap(forward, in_axes=in_axes, devices=devs))
    else:
        _fwd = ('jit', jax.jit(jax.vmap(forward, in_axes=(0,) + (None,) * 27)))
    _fwd = _fwd + (jax.jit(jax.vmap(forward, in_axes=(0,) + (None,) * 27)),)


def kernel(**inputs):
    _build()
    wf = np.asarray(inputs['waveform'], np.float32).reshape(B, 1, SAMPLES)
    names = ['mel_fb', 'window', 'conv1_w', 'conv1_b', 'conv2_w', 'conv2_b',
             'proj_w', 'proj_b', 'query_embed', 'cls_w', 'cls_b',
             'enc_attn_w', 'enc_attn_b', 'enc_ln', 'enc_ff_w1', 'enc_ff_b1',
             'enc_ff_w2', 'enc_ff_b2',
             'dec_sa_w', 'dec_sa_b', 'dec_ca_w', 'dec_ca_b', 'dec_ln',
             'dec_ff_w1', 'dec_ff_b1', 'dec_ff_w2', 'dec_ff_b2']
    params = [np.asarray(inputs[n], np.float32) for n in names]
    global _mode
    try:
        out, spec, backbone = _fwd[1](wf, *params)
    except Exception:
        out, spec, backbone = _fwd[2](wf, *params)
    logits = np.asarray(out).reshape(B, TT, NC)
    spec = np.asarray(spec).reshape(B, 1, NFRAMES, N_MELS)
    backbone = np.asarray(backbone).reshape(B, CH, NFRAMES, F_OUT)
    return logits, spec, backbone
